# revision 33
# baseline (speedup 1.0000x reference)
"""CenterLoss Trainium2 kernel (8-core SPMD, data-parallel over batch).

loss = mean_i( ||feat_i - centers[label_i]|| / count[label_i] )

Device algorithm (per core, batch shard of 2048 rows, fp8_e4m3 staging):
  - feat/centers staged in DRAM as fp8_e4m3 (rel quantization error on the
    loss ~4e-4, far inside the 2e-2 gate).
  - centers[label] gathered in four 512-row quarters on four parallel SWDGE
    queues (single-queue Q7 descriptor generation is ~14us; ~3.7us on 4
    queues). feat streamed by ONE plain HWDGE load (~350GB/s; each extra
    dma_start call costs ~0.7-1.2us serialized, so calls are minimized).
    The old accumulating-CCE subtract (~1.9us/call on the Pool ring, any
    size) was the hidden bottleneck and is gone: the subtract is fused
    into the DVE square op instead.
  - dist2_i = sum_d (f-c)^2: all 16 128x512 tiles via a fused custom DVE
    op SQDIFF_REDUCE_ANT (out=(a-b)^2, accum_out=rowsum). DVE runs
    1 elem/cycle/lane (~0.47us/tile) and is the kernel's roofline; every
    other engine hides under it.
  - radix-100 class factorization: c = 100*h + l; one-hot encodings
    A[i,h], B[i,l] are host-precomputed (pure label encodings, like gidx)
    and uploaded as one [P,2R,T] bf16 tensor into a [P,3R,T] tile whose
    last R columns hold B*dist. B*dist runs on ACT via the per-partition
    scale port (16 small scalar.mul), so DVE carries nothing but the
    squares. A single 16-matmul PE group accumulates the histogram
    cnt2d[h,l] and dist sums S2d[h,l] into one [R,2R] PSUM tile.
  - no cross-rep serializers: no scalar chain op (each rep's own output DMA
    keeps it live under DCE; reps>1 exist only for timing and write the
    same bytes), tail ops (psum copy + out DMA) on ACT only, DVE/Pool/SP
    carry only early/mid-pipeline work so consecutive reps overlap.
  - outputs in fp16 (counts <= 2048 and bin sums exactly/safely
    representable), one [R,2R] DMA per rep.
  - host: cnt = sum_k cnt_k, S = sum_k S_k, loss = sum(S/max(cnt,1))/B.
"""

from contextlib import ExitStack
from operator import add

import numpy as np

import concourse.bass as bass
import concourse.tile as tile
from concourse import bacc, mybir
from concourse import bass_utils
from concourse.alu_op_type import AluOpType

B, D, C = 16384, 512, 10000
NCORES = 8
BLOC = B // NCORES  # 2048 rows per core
P = 128
TLOC = BLOC // P    # 16 local batch tiles
R = 100             # radix (c = 100*h + l)
NDVE = 16           # square-reduce tiles on DVE fused op; rest DVE-sub+ACT-sq

F32 = mybir.dt.float32
F16 = mybir.dt.float16
BF16 = mybir.dt.bfloat16
I16 = mybir.dt.int16
F8 = mybir.dt.float8e4
F8NP = mybir.dt.np(F8)
BF16NP = mybir.dt.np(BF16)

_CACHE: dict = {}


def _register_custom_op(name, body_fn, ref, rd1_en):
    """Register a custom DVE op with accum-rowsum (idempotent)."""
    from concourse import dve_ops
    from concourse.dve_spec import Spec, Zero, lower
    from concourse.dve_uop import DveOpSpec

    if name in dve_ops._SUB_OPCODE_FOR_NAME:
        return next(op for op in dve_ops.OPS if op.name == name)
    spec = Spec(body=body_fn(), accum=add, accum_init=Zero, reference=ref)
    row = max(dve_ops._SUB_OPCODE_FOR_NAME.values()) + 1
    assert row < 0x20
    shas = {
        ver: DveOpSpec(
            name=name, opcode=row, uops=lower(spec, ver=ver), rd1_en=rd1_en
        ).sha(ver)
        for ver in ("v3", "v4")
    }
    op = dve_ops.DveOp(name, spec, subdim=False, uops_sha=shas)
    dve_ops.OPS.append(op)
    dve_ops._SUB_OPCODE_FOR_NAME[name] = row
    dve_ops.CUSTOM_DVE_SPECS[name] = spec
    return op


def _register_sqdiff():
    from concourse.dve_spec import Src0, Src1, sq

    def _ref(in0, in1, s0, s1, imm2):
        b = (in0.astype(np.float32) - in1.astype(np.float32)) ** 2
        return b, b.reshape(b.shape[0], -1).sum(-1, keepdims=True)

    return _register_custom_op(
        "SQDIFF_REDUCE_ANT", lambda: sq(Src0 - Src1), _ref, True
    )


def build_program(
    reps: int = 1,
    variant: str = "full",
    ndve: int | None = None,
    ncce: int = 0,
    sqmode: str = "custom",
    bbact: bool = True,
    nloads: int = 1,
    sq32: bool = False,
    sqpair: int = 1,
    tailq: bool = False,
):
    """Build + compile the per-core Bass program (SPMD: same program on
    all 8 cores, different input data).

    reps > 1 repeats the whole body; each rep writes the (same) output so
    DCE keeps it. Marginal wall-clock per rep = pure device time.

    ncce top tiles (whole quarters: 0/4/8) get their diff via CCE
    accumulating-subtract DMA (fp8) and are squared on ACT; the rest run
    the fused DVE sqdiff op.

    variant selects a subset of the pipeline for HW stage attribution:
      full | gather | load | load4 | dma (gather+load) | ohload | compute
    """
    if ndve is None:
        ndve = NDVE
    ncce = 0  # CCE offload measured strictly worse on HW; path removed
    ntv = TLOC - ncce  # tiles on the DVE sqdiff path
    sqdiff_op = _register_sqdiff()
    nc = bacc.Bacc(
        "TRN2",
        target_bir_lowering=False,
        debug=False,
        enable_asserts=False,
        num_swdge_queues=4,
    )

    feat_d = nc.dram_tensor("feat8", [BLOC, D], F8, kind="ExternalInput").ap()
    cent_d = nc.dram_tensor("cent8", [C, D], F8, kind="ExternalInput").ap()
    gidx_d = nc.dram_tensor("gidx", [P, BLOC // 16], I16, kind="ExternalInput").ap()
    ab_d = nc.dram_tensor("ab_oh", [P, 2 * R, TLOC], BF16, kind="ExternalInput").ap()
    cs_out_d = nc.dram_tensor("cs_out", [R, 2 * R], F16, kind="ExternalOutput").ap()
    dbg_d = nc.dram_tensor("dbg", [1, 64], F8, kind="ExternalOutput").ap()

    feat_r = feat_d.rearrange("(p t) d -> p t d", p=P)

    do_gather = variant in ("full", "gather", "dma")
    do_load = variant in ("full", "load", "load4", "dma")
    do_ohload = variant in ("full", "ohload", "compute")
    do_compute = variant in ("full", "compute")

    with tile.TileContext(nc) as tc, ExitStack() as ctx:
        const = ctx.enter_context(tc.tile_pool(name="const", bufs=4))
        big = ctx.enter_context(tc.tile_pool(name="big", bufs=3))
        work = ctx.enter_context(tc.tile_pool(name="work", bufs=4))
        fin = ctx.enter_context(tc.tile_pool(name="fin", bufs=3))
        psum = ctx.enter_context(tc.tile_pool(name="psum", bufs=6, space="PSUM"))

        Q = TLOC // 4
        for _rep in range(reps):
            if do_gather:
                gidx_s = const.tile([P, BLOC // 16], I16, tag="gidx")
                nc.sync.dma_start(gidx_s[:], gidx_d[:])
                cent_s = big.tile([P, TLOC, D], F8, tag="cent")
                for q in range(4):
                    nc.gpsimd.dma_gather(
                        out_ap=cent_s[:, q * Q : (q + 1) * Q],
                        in_ap=cent_d[:],
                        idxs_ap=gidx_s[:, q * (BLOC // 64) : (q + 1) * (BLOC // 64)],
                        num_idxs=BLOC // 4,
                        num_idxs_reg=BLOC // 4,
                        elem_size=D,
                        single_packet=False,
                        queue_num=q,
                    )
            if do_load:
                feat_s = big.tile([P, ntv, D], F8, tag="feat")
                nl = 4 if variant == "load4" else nloads
                for i in range(nl):
                    sl = slice(i * (ntv // nl), (i + 1) * (ntv // nl))
                    nc.sync.dma_start(feat_s[:, sl], feat_r[:, sl])
            if do_ohload:
                abt = fin.tile([P, 3 * R, TLOC], BF16, tag="abt")
                nc.sync.dma_start(abt[:, : 2 * R], ab_d[:])

            if do_compute:
                if variant == "compute":
                    # stand-alone compute: tiny loads; square ops re-read
                    # tile 0 (timing only; element counts unchanged)
                    cent_s = big.tile([P, 1, D], F8, tag="cent")
                    nc.sync.dma_start(cent_s[:, 0:1], feat_r[:, 1:2])
                    feat_s = big.tile([P, 1, D], F8, tag="feat")
                    nc.sync.dma_start(feat_s[:, 0:1], feat_r[:, 0:1])

                dist2 = fin.tile([P, TLOC], F32, tag="dist2")
                dist_bf = fin.tile([P, TLOC], F32, tag="dist_bf")
                if sq32:
                    dist2h = fin.tile([P, 2 * TLOC], F32, tag="dist2h")
                if sqpair > 1:
                    # fewer, wider DVE instructions (no accum port: it is
                    # [P,1]-only); row-sums go to ACT's accumulate port
                    assert variant == "full" and ntv % sqpair == 0
                    for t0 in range(0, ntv, sqpair):
                        scr = work.tile([P, sqpair, D], BF16, tag="sqscr")
                        nc.vector._custom_dve(
                            sqdiff_op,
                            out=scr[:],
                            in0=feat_s[:, t0 : t0 + sqpair],
                            in1=cent_s[:, t0 : t0 + sqpair],
                        )
                        for dt in range(sqpair):
                            s2 = work.tile([P, D], BF16, tag="sqact")
                            nc.scalar.activation(
                                s2[:],
                                scr[:, dt],
                                mybir.ActivationFunctionType.Copy,
                                accum_out=dist2[:, t0 + dt : t0 + dt + 1],
                            )
                for t in range(TLOC):
                    if sqpair > 1:
                        break
                    tin = 0 if variant == "compute" else t
                    if t < ntv:
                        if sq32:
                            for hh in range(2):
                                scr = work.tile([P, D // 2], BF16, tag="sqscr")
                                nc.vector._custom_dve(
                                    sqdiff_op,
                                    out=scr[:],
                                    in0=feat_s[:, tin, hh * 256 : hh * 256 + 256],
                                    in1=cent_s[:, tin, hh * 256 : hh * 256 + 256],
                                    accum_out=dist2h[:, 2 * t + hh : 2 * t + hh + 1],
                                )
                            if t == TLOC - 1:
                                nc.vector.tensor_tensor(
                                    dist2[:],
                                    dist2h[:, 0 : 2 * TLOC : 2],
                                    dist2h[:, 1 : 2 * TLOC : 2],
                                    AluOpType.add,
                                )
                            continue
                        scr = work.tile([P, D], BF16, tag="sqscr")
                        nc.vector._custom_dve(
                            sqdiff_op,
                            out=scr[:],
                            in0=feat_s[:, tin],
                            in1=cent_s[:, tin],
                            accum_out=dist2[:, t : t + 1],
                        )
                # dist (bf16) + dist-scaled one-hot into abt[:, 2R:], then
                # one 16-matmul PE group accumulating [cnt2d | S2d]
                psum_cs = psum.tile([R, 2 * R], F32, tag="psum_cs")
                ntail = 4 if tailq else 2
                H = TLOC // ntail
                for h in range(ntail):
                    sl = slice(h * H, (h + 1) * H)
                    nc.scalar.activation(
                        dist_bf[:, sl],
                        dist2[:, sl],
                        mybir.ActivationFunctionType.Sqrt,
                    )
                    if not bbact:
                        nc.vector.tensor_tensor(
                            abt[:, 2 * R :, sl],
                            abt[:, R : 2 * R, sl],
                            dist_bf[:, sl].unsqueeze(1).broadcast_to([P, R, H]),
                            AluOpType.mult,
                        )
                    for j in range(H):
                        t = h * H + j
                        if bbact:
                            # B*dist on ACT via the per-partition scale port
                            # (keeps DVE free for the square-reduce stream)
                            nc.scalar.mul(
                                abt[:, 2 * R :, t],
                                abt[:, R : 2 * R, t],
                                dist_bf[:, t : t + 1],
                            )
                        nc.tensor.matmul(
                            psum_cs[:],
                            abt[:, :R, t],
                            abt[:, R:, t],
                            start=(t == 0),
                            stop=(t == TLOC - 1),
                        )

                cs_sb = fin.tile([R, 2 * R], F16, tag="cs_sb")
                nc.scalar.copy(cs_sb[:], psum_cs[:])
                # per-rep output DMA (side effect keeps the rep live; all
                # reps write identical bytes, so racing writes are benign)
                nc.scalar.dma_start(cs_out_d[:], cs_sb[:])
            else:
                # DMA-only variants: tiny probe DMAs spanning every
                # transfer of this rep keep it live
                if do_gather:
                    nc.sync.dma_start(dbg_d[0:1, 0:TLOC], cent_s[0:1, :, 0])
                if do_load:
                    nc.sync.dma_start(
                        dbg_d[0:1, TLOC : 2 * TLOC], feat_s[0:1, :, 0]
                    )
                if do_ohload:
                    prb = work.tile([1, 8], F8, tag="prb")
                    nc.vector.tensor_copy(prb[:], abt[0:1, 0 : 2 * R : 25, 0])
                    nc.sync.dma_start(dbg_d[0:1, 32:40], prb[:])

    nc.compile()
    return nc


def make_in_maps(feat, label, centers):
    """Shard + lay out full inputs into the 8 per-core input maps."""
    feat = np.asarray(feat, dtype=np.float32)
    label = np.asarray(label, dtype=np.int32)
    centers = np.asarray(centers, dtype=np.float32)
    feat8 = feat.astype(F8NP)
    cent8 = np.ascontiguousarray(centers.astype(F8NP))

    g = np.arange(BLOC)
    perm = (g % P) * TLOC + (g // P)  # gather order -> local row index
    eye = np.eye(R, dtype=BF16NP)

    in_maps = []
    for k in range(NCORES):
        lab_k = label[k * BLOC : (k + 1) * BLOC]
        gvals = lab_k[perm].astype(np.int16)  # idx list in gather order
        gidx16 = np.ascontiguousarray(gvals.reshape(BLOC // 16, 16).T)  # [16, 128]
        gidx = np.ascontiguousarray(np.tile(gidx16, (P // 16, 1)))
        lab2 = lab_k.reshape(P, TLOC)
        a_oh = eye[lab2 // R].transpose(0, 2, 1)  # [P, R, T]
        b_oh = eye[lab2 % R].transpose(0, 2, 1)
        ab = np.ascontiguousarray(np.concatenate([a_oh, b_oh], axis=1))
        in_maps.append(
            {
                "feat8": np.ascontiguousarray(feat8[k * BLOC : (k + 1) * BLOC]),
                "cent8": cent8,
                "gidx": gidx,
                "ab_oh": ab,
            }
        )
    return in_maps


def get_program():
    if "nc" not in _CACHE:
        _CACHE["nc"] = build_program()
    return _CACHE["nc"]


def kernel(feat, label, centers):
    nc = get_program()
    in_maps = make_in_maps(feat, label, centers)
    res = bass_utils.run_bass_kernel_spmd(nc, in_maps, core_ids=list(range(NCORES)))
    s_tot = np.zeros((R, R), dtype=np.float64)
    c_tot = np.zeros((R, R), dtype=np.float64)
    for k in range(NCORES):
        cs = res.results[k]["cs_out"].astype(np.float64)
        c_tot += cs[:, :R]
        s_tot += cs[:, R:]
    loss = (s_tot / np.maximum(c_tot, 1.0)).sum() / B
    return np.asarray(loss, dtype=np.float32)
